# revision 1
# baseline (speedup 1.0000x reference)
"""v6: per-slot bf16 matmul DHG kernel — no DRAM tables, no dma_gather.

Host lays out transposed per-slot features (fp8) so the phase-A matmul
output lands directly in the phase-B (group-per-partition) layout.
"""
import numpy as np
import ml_dtypes
import concourse.bass as bass
import concourse.bacc as bacc
import concourse.tile as tile
from concourse import mybir

P = 128
NM = 5               # megatiles per core
KAP = 4              # edges per partition per megatile
GPP = KAP * 5        # 20 groups per partition
NIW = GPP * 8        # 160 slot-rows per partition
NSL = NIW * P        # 20480 slots per megatile
BSZ = 4096           # slots per DMA batch
NBT = NSL // BSZ     # 10 batches per megatile
RS = 40              # record stride cols (q,k,v,G32,F2,pad3)
G8 = RS * 8          # group stride in record cols
TC = 40              # wcat cols
EPC = NM * P * KAP   # 2560 edges/core padded

bf = mybir.dt.bfloat16
f32 = mybir.dt.float32
f8 = mybir.dt.float8e4
MUL = mybir.AluOpType.mult
ADD = mybir.AluOpType.add
MAX = mybir.AluOpType.max
AF = mybir.ActivationFunctionType
X = mybir.AxisListType.X


def ap_of(t, off, dims):
    return bass.AP(tensor=t.tensor, offset=t.offset + off, ap=[list(t.ap[0])] + [list(d) for d in dims])


def build(n_cores=8, repeat=1, mode="full", loop_n=None, fdt=None):
    FDT = fdt or bf
    nc = bacc.Bacc("TRN2", target_bir_lowering=False, debug=False, num_devices=n_cores)
    fTC = nc.declare_dram_parameter("fTC", [NM, NBT, P, BSZ], FDT, isOutput=False)
    wcat_d = nc.declare_dram_parameter("wcat", [P, TC], bf, isOutput=False)
    cb_d = nc.declare_dram_parameter("consts", [P, 66], f32, isOutput=False)
    cbh_d = nc.declare_dram_parameter("constsh", [P, 64], bf, isOutput=False)
    out_d = nc.declare_dram_parameter("out", [NM, P, KAP * 2], f32, isOutput=True)

    with tile.TileContext(nc) as tc:
        with tc.tile_pool(name="cons", bufs=1) as cons, \
             tc.tile_pool(name="pa", bufs=4) as pa, \
             tc.tile_pool(name="pap", bufs=4, space="PSUM") as pap, \
             tc.tile_pool(name="pb", bufs=2) as pb:
            wcat_t = cons.tile([P, TC], bf)
            nc.sync.dma_start(out=wcat_t[:], in_=wcat_d[:])
            cb_t = cons.tile([P, 66], f32)
            nc.sync.dma_start(out=cb_t[:], in_=cb_d[:])
            cbh_t = cons.tile([P, 64], bf)
            nc.sync.dma_start(out=cbh_t[:], in_=cbh_d[:])

            def phase_a(m):
                """DMA per-slot transposed feats, matmul vs wcat, records into SBUF."""
                rec = pb.tile([P, NIW * RS], bf, tag="rec")
                for b in range(NBT):
                    lhsT = pa.tile([P, BSZ], FDT, tag="lhsT")
                    nc.sync.dma_start(out=lhsT[:], in_=fTC[m][b])
                    if mode == "dmaonly":
                        continue
                    for q in range(2):
                        ps = pap.tile([P, 1024], f32)
                        for h in range(2):
                            for c in range(8):
                                ci = q * 16 + h * 8 + c
                                nc.tensor.matmul(out=ps[:, h * 512 + c * TC:h * 512 + (c + 1) * TC],
                                                 lhsT=lhsT[:, ci * P:(ci + 1) * P],
                                                 rhs=wcat_t[:], start=True, stop=True)
                        if mode == "nocopy":
                            continue
                        nc.scalar.copy(out=rec[:, (b * 2 + q) * 640:(b * 2 + q + 1) * 640],
                                       in_=ap_of(ps, 0, [(512, 2), (1, 320)]))
                return rec

            ufall = cons.tile([P, NM * GPP * 34], bf)

            def phase_b(m, rec):
                """Attention head per megatile: dg gate + dg-weighted G32/F2
                sums into ufall[m]. Thin per-group tail deferred to tail()."""
                S = pb.tile([P, GPP * 64], bf, tag="S")
                nc.vector.tensor_tensor(
                    out=ap_of(S, 0, [(64, GPP), (8, 8), (1, 8)]),
                    in0=ap_of(rec, 0, [(G8, GPP), (RS, 8), (0, 8)]),
                    in1=ap_of(rec, 1, [(G8, GPP), (0, 8), (RS, 8)]), op=MUL)
                nc.vector.memset(ap_of(S, 0, [(64, GPP), (9, 8)]), -88.0)
                # ET: per group [E(8x8) | tv(8x8)] so one tree reduces both
                ET = pb.tile([P, GPP * 128], bf, tag="ET")
                nc.scalar.activation(out=ap_of(ET, 0, [(128, GPP), (1, 64)]),
                                     in_=S[:], func=AF.Exp)
                nc.vector.tensor_tensor(
                    out=ap_of(ET, 64, [(128, GPP), (8, 8), (1, 8)]),
                    in0=ap_of(ET, 0, [(128, GPP), (8, 8), (1, 8)]),
                    in1=ap_of(rec, 2, [(G8, GPP), (0, 8), (RS, 8)]), op=MUL)
                Q4 = pb.tile([P, GPP * 64], bf, tag="Q4")
                nc.vector.tensor_tensor(
                    out=ap_of(Q4, 0, [(64, GPP), (4, 16), (1, 4)]),
                    in0=ap_of(ET, 0, [(128, GPP), (8, 16), (1, 4)]),
                    in1=ap_of(ET, 4, [(128, GPP), (8, 16), (1, 4)]), op=ADD)
                Q2 = pb.tile([P, GPP * 32], bf, tag="Q2")
                nc.vector.tensor_tensor(
                    out=ap_of(Q2, 0, [(32, GPP), (2, 16), (1, 2)]),
                    in0=ap_of(Q4, 0, [(64, GPP), (4, 16), (1, 2)]),
                    in1=ap_of(Q4, 2, [(64, GPP), (4, 16), (1, 2)]), op=ADD)
                rsts = pb.tile([P, GPP * 16], f32, tag="rsts")
                nc.vector.tensor_tensor(
                    out=ap_of(rsts, 0, [(16, GPP), (1, 16)]),
                    in0=ap_of(Q2, 0, [(32, GPP), (2, 16)]),
                    in1=ap_of(Q2, 1, [(32, GPP), (2, 16)]), op=ADD)
                rv = pb.tile([P, NIW], f32, tag="rv")
                nc.vector.reciprocal_approx_fast(
                    out=ap_of(rv, 0, [(8, GPP), (1, 8)]),
                    in_=ap_of(rsts, 0, [(16, GPP), (1, 8)]))
                td = pb.tile([P, NIW], f32, tag="td")
                nc.vector.tensor_tensor(
                    out=ap_of(td, 0, [(8, GPP), (1, 8)]),
                    in0=ap_of(rsts, 8, [(16, GPP), (1, 8)]),
                    in1=ap_of(rv, 0, [(8, GPP), (1, 8)]), op=MUL)
                dg = pb.tile([P, NIW], bf, tag="dg")
                nc.scalar.activation(out=dg[:], in_=td[:], func=AF.Tanh)
                # dg duplicated into pairs: all three prodC operands iterate
                # (slot, colpair, 2) with innermost step 1 -> DVE 2x_1P mode
                dgp = pb.tile([P, NIW * 2], bf, tag="dgp")
                nc.vector.tensor_copy(
                    out=ap_of(dgp, 0, [(2, NIW), (1, 2)]),
                    in_=ap_of(dg, 0, [(1, NIW), (0, 2)]))
                # dg-weighted 34-wide records (G32|F2 at col 4) + pairwise j-tree
                prodC = pb.tile([P, GPP * 8 * 34], bf, tag="prodC")
                nc.vector.tensor_tensor(
                    out=ap_of(prodC, 0, [(34, NIW), (2, 17), (1, 2)]),
                    in0=ap_of(rec, 4, [(RS, NIW), (2, 17), (1, 2)]),
                    in1=ap_of(dgp, 0, [(2, NIW), (0, 17), (1, 2)]), op=MUL)
                C1t = pb.tile([P, GPP * 4 * 34], bf, tag="C1t")
                nc.vector.tensor_tensor(
                    out=ap_of(C1t, 0, [(136, GPP), (34, 4), (1, 34)]),
                    in0=ap_of(prodC, 0, [(272, GPP), (68, 4), (1, 34)]),
                    in1=ap_of(prodC, 34, [(272, GPP), (68, 4), (1, 34)]), op=ADD)
                C2t = pb.tile([P, GPP * 2 * 34], bf, tag="C2t")
                nc.vector.tensor_tensor(
                    out=ap_of(C2t, 0, [(68, GPP), (34, 2), (1, 34)]),
                    in0=ap_of(C1t, 0, [(136, GPP), (68, 2), (1, 34)]),
                    in1=ap_of(C1t, 34, [(136, GPP), (68, 2), (1, 34)]), op=ADD)
                nc.vector.tensor_tensor(
                    out=ap_of(ufall, m * GPP * 34, [(34, GPP), (1, 34)]),
                    in0=ap_of(C2t, 0, [(68, GPP), (1, 34)]),
                    in1=ap_of(C2t, 34, [(68, GPP), (1, 34)]), op=ADD)

            def tail():
                """Per-group MLP score + softmax over 5 + sigmoid head,
                batched across all NM megatiles (one pass, few big ops)."""
                NG5 = NM * GPP               # 100 groups per partition
                ub = pb.tile([P, NG5 * 32], bf, tag="ub")
                nc.vector.tensor_tensor(
                    out=ap_of(ub, 0, [(32, NG5), (1, 32)]),
                    in0=ap_of(ufall, 0, [(34, NG5), (1, 32)]),
                    in1=ap_of(cbh_t, 0, [(0, NG5), (1, 32)]), op=ADD)
                rl = pb.tile([P, NG5 * 32], bf, tag="rl")
                nc.scalar.activation(out=rl[:], in_=ub[:], func=AF.Relu)
                wm = pb.tile([P, NG5 * 32], bf, tag="wm")
                nc.vector.tensor_tensor(
                    out=ap_of(wm, 0, [(32, NG5), (1, 32)]),
                    in0=ap_of(rl, 0, [(32, NG5), (1, 32)]),
                    in1=ap_of(cbh_t, 32, [(0, NG5), (1, 32)]), op=MUL)
                T1 = pb.tile([P, NG5 * 16], bf, tag="T1")
                nc.vector.tensor_tensor(
                    out=ap_of(T1, 0, [(16, NG5), (1, 16)]),
                    in0=ap_of(wm, 0, [(32, NG5), (1, 16)]),
                    in1=ap_of(wm, 16, [(32, NG5), (1, 16)]), op=ADD)
                T2 = pb.tile([P, NG5 * 8], bf, tag="T2")
                nc.vector.tensor_tensor(
                    out=ap_of(T2, 0, [(8, NG5), (1, 8)]),
                    in0=ap_of(T1, 0, [(16, NG5), (1, 8)]),
                    in1=ap_of(T1, 8, [(16, NG5), (1, 8)]), op=ADD)
                T3 = pb.tile([P, NG5 * 4], bf, tag="T3")
                nc.vector.tensor_tensor(
                    out=ap_of(T3, 0, [(4, NG5), (1, 4)]),
                    in0=ap_of(T2, 0, [(8, NG5), (1, 4)]),
                    in1=ap_of(T2, 4, [(8, NG5), (1, 4)]), op=ADD)
                T4 = pb.tile([P, NG5 * 2], bf, tag="T4")
                nc.vector.tensor_tensor(
                    out=ap_of(T4, 0, [(2, NG5), (1, 2)]),
                    in0=ap_of(T3, 0, [(4, NG5), (1, 2)]),
                    in1=ap_of(T3, 2, [(4, NG5), (1, 2)]), op=ADD)
                scall = pb.tile([P, NG5], f32, tag="scall")
                nc.vector.tensor_tensor(
                    out=ap_of(scall, 0, [(1, NG5)]),
                    in0=ap_of(T4, 0, [(2, NG5)]),
                    in1=ap_of(T4, 1, [(2, NG5)]), op=ADD)
                esc = pb.tile([P, NG5], f32, tag="esc")
                nc.scalar.activation(out=esc[:], in_=scall[:], func=AF.Exp)
                ssum = pb.tile([P, NM * KAP], f32, tag="ssum")
                nc.vector.tensor_reduce(out=ssum[:], in_=ap_of(esc, 0, [(5, NM * KAP), (1, 5)]),
                                        axis=X, op=ADD)
                sr = pb.tile([P, NM * KAP], f32, tag="sr")
                nc.vector.reciprocal_approx_fast(out=sr[:], in_=ssum[:])
                ha = pb.tile([P, NM * KAP * 10], f32, tag="ha")
                nc.vector.tensor_tensor(
                    out=ap_of(ha, 0, [(10, NM * KAP), (5, 2), (1, 5)]),
                    in0=ap_of(ufall, 32, [(170, NM * KAP), (1, 2), (34, 5)]),
                    in1=ap_of(esc, 0, [(5, NM * KAP), (0, 2), (1, 5)]), op=MUL)
                lo = pb.tile([P, NM * KAP * 2], f32, tag="lo")
                nc.vector.tensor_reduce(out=lo[:], in_=ap_of(ha, 0, [(10, NM * KAP), (5, 2), (1, 5)]),
                                        axis=X, op=ADD)
                lon = pb.tile([P, NM * KAP * 2], f32, tag="lon")
                nc.vector.tensor_tensor(
                    out=ap_of(lon, 0, [(2, NM * KAP), (1, 2)]),
                    in0=ap_of(lo, 0, [(2, NM * KAP), (1, 2)]),
                    in1=ap_of(sr, 0, [(1, NM * KAP), (0, 2)]), op=MUL)
                lb = pb.tile([P, NM * KAP * 2], f32, tag="lb")
                nc.vector.tensor_tensor(out=lb[:], in0=lon[:],
                                        in1=ap_of(cb_t, 64, [(0, NM * KAP), (1, 2)]), op=ADD)
                # sigmoid(x) = 0.5*tanh(0.5x)+0.5 — stays in exp_and_others set
                th = pb.tile([P, NM * KAP * 2], f32, tag="th")
                nc.scalar.activation(out=th[:], in_=lb[:], func=AF.Tanh, scale=0.5)
                ov = pb.tile([P, NM * KAP * 2], f32, tag="ov")
                nc.vector.tensor_scalar(out=ov[:], in0=th[:], scalar1=0.5, scalar2=0.5,
                                        op0=MUL, op1=ADD)
                for m in range(NM):
                    nc.sync.dma_start(out=out_d[m], in_=ov[:, m * KAP * 2:(m + 1) * KAP * 2])

            def one_pass():

                # emit phase_b(m) BEFORE phase_a(m+1): per-engine queues are
                # FIFO, so phase_b's ACT ops (exp/tanh) must not queue behind
                # the next megatile's 20 PSUM->SBUF copies
                rec = phase_a(0)
                for m in range(NM):
                    if mode == "full":
                        phase_b(m, rec)
                    rec = phase_a(m + 1) if m + 1 < NM else None
                if mode == "full":
                    tail()

            if loop_n is not None:
                with tc.For_i(0, loop_n):
                    for _rep in range(repeat):
                        one_pass()
            else:
                for _rep in range(repeat):
                    one_pass()
    nc.compile()
    return nc


HOST_FDT = ml_dtypes.bfloat16


def host_prepare(feats, edge_members, adj_members, wq, wk, wv, W1, b1, W2, Wfc, bfc, n_cores=8):
    V, D = feats.shape
    E = edge_members.shape[0]
    epc_real = E // n_cores
    mem_all = np.concatenate([edge_members[:, None, :], adj_members], axis=1).astype(np.int64)  # [E,5,8]

    wcat = np.zeros((D, TC), np.float32)
    wcat[:, 0] = wq[:, 0]; wcat[:, 1] = wk[:, 0]; wcat[:, 2] = wv[:, 0]
    wcat[:, 4:36] = W1; wcat[:, 36:38] = Wfc
    wcat = wcat.astype(ml_dtypes.bfloat16)
    cb = np.zeros((P, 66), np.float32)
    cb[:, 0:32] = b1[None, :]; cb[:, 32:64] = W2[:, 0][None, :]; cb[:, 64:66] = bfc[None, :]
    cbh = cb[:, :64].astype(ml_dtypes.bfloat16)
    feats_f8 = np.asarray(feats, np.float32).astype(HOST_FDT)

    in_maps = []
    for c in range(n_cores):
        el = np.zeros((EPC,), np.int64)
        el[:epc_real] = np.arange(c * epc_real, (c + 1) * epc_real)
        mem = mem_all[el].reshape(NM, P, KAP, 5, 8)   # edge (m,p,k) = m*512 + p*4 + k
        fTC = np.zeros((NM, NBT, P, BSZ), HOST_FDT)
        for m in range(NM):
            # slot (p, srow) at column srow*128+p; srow = k*40 + cc*8 + j
            V2 = mem[m].transpose(1, 2, 3, 0).reshape(NIW * P)   # [srow, p] flat
            fTC[m] = feats_f8[V2].T.reshape(P, NBT, BSZ).transpose(1, 0, 2)
        in_maps.append({"fTC": fTC, "wcat": wcat, "consts": cb, "constsh": cbh})

    def unpack(results):
        outs = []
        for c in range(n_cores):
            o = results[c]["out"].reshape(NM, P, KAP, 2).reshape(EPC, 2)[:epc_real]
            outs.append(o)
        return np.concatenate(outs, axis=0)
    return in_maps, unpack


# ------------------------------------------------------------------
# Public entry point: kernel(**inputs) -> [20000, 2] float32
# ------------------------------------------------------------------
from concourse.bass_utils import run_bass_kernel_spmd

_CACHED_NC = None

def kernel(feats, edge_members, adj_members, ids, epoch,
           wq, bq, wk, bk, wv, bv, W1, b1, W2, b2, Wfc, bfc):
    """DHGLayerV1 forward on 8 NeuronCores.

    Strategy: edges sharded across 8 cores (2500 each), 5 megatiles per
    core. Host lays out bf16 feats transposed per SLOT (one column per
    group-member occurrence, ordered so the phase-A matmul against
    bf16 [wq|wk|wv|W1|Wfc] lands records (q,k,v,G32,F2) directly in the
    group-per-partition phase-B layout). Phase B (masked softmax over
    K=8, tanh gate, dg-weighted G32/F2 sums, relu-MLP score, softmax
    over 5 candidates, sigmoid head) runs on DVE/ACT. b2 is dropped
    (softmax-invariant); bq/bk/bv asserted zero; b1/bfc applied exactly.
    """
    global _CACHED_NC
    feats = np.asarray(feats, dtype=np.float32)
    edge_members = np.asarray(edge_members)
    adj_members = np.asarray(adj_members)
    wq = np.asarray(wq, np.float32); wk = np.asarray(wk, np.float32)
    wv = np.asarray(wv, np.float32); W1 = np.asarray(W1, np.float32)
    b1 = np.asarray(b1, np.float32); W2 = np.asarray(W2, np.float32)
    Wfc = np.asarray(Wfc, np.float32); bfc = np.asarray(bfc, np.float32)
    assert np.all(np.asarray(bq) == 0) and np.all(np.asarray(bk) == 0) \
        and np.all(np.asarray(bv) == 0), "nonzero q/k/v biases unsupported"

    if _CACHED_NC is None:
        _CACHED_NC = build(n_cores=8)
    nc = _CACHED_NC
    in_maps, unpack = host_prepare(feats, edge_members, adj_members,
                                   wq, wk, wv, W1, b1, W2, Wfc, bfc, n_cores=8)
    res = run_bass_kernel_spmd(nc, in_maps, core_ids=list(range(8)))
    return unpack(res.results).astype(np.float32)

